# revision 7
# baseline (speedup 1.0000x reference)
"""TRN2 Bass kernel for nn_Encoder_60112362275061 (GRU encoder).

B=128, T=1024, X=256, H=512 GRU; returns final hidden state h_T [B, H].
Data-parallel over 8 NeuronCores (16 batch rows per core); weights
replicated. See build_kernel() docstring for the per-core design.

Self-contained: hardcodes shapes/sharding; only imports the container
toolchain (concourse) and numpy.
"""

import sys

for _p in ("/opt/trn_rl_repo",):
    if _p not in sys.path:
        sys.path.insert(0, _p)

import numpy as np

import concourse.bass as bass
import concourse.mybir as mybir
from concourse.tile import TileContext

F32 = mybir.dt.float32

B, T_FULL, X, H = 128, 1024, 256, 512
NCORES = 8
BS = B // NCORES          # 16 batch rows per core
NG = 4                    # psum column groups == h chunks
HC = H // NG              # 128 h dims per chunk
GFD = 3 * HC              # 384 weight cols per group [r_j|z_j|n_j]
PB = 4 * HC               # 512 psum cols per step [r|z|hn|xn]
CH = 32                   # timesteps per For_i iteration


def gate_perm():
    """Permutation P of the 3H gate dim: group j gets [r_j | z_j | n_j]."""
    idx = []
    for j in range(NG):
        idx.extend(range(j * HC, (j + 1) * HC))                  # r_j
        idx.extend(range(H + j * HC, H + (j + 1) * HC))          # z_j
        idx.extend(range(2 * H + j * HC, 2 * H + (j + 1) * HC))  # n_j
    return np.array(idx)


def host_prepare_weights(W_ih, W_hh, b_ih, b_hh):
    """Build device weight tensors (shared by all cores)."""
    P = gate_perm()
    wih = np.ascontiguousarray(W_ih.T[:, P]).astype(np.float32)  # [256, 1536]
    whh = np.ascontiguousarray(W_hh.T[:, P]).astype(np.float32)  # [512, 1536]
    bih_p = b_ih[P].astype(np.float32)
    bhh_p = b_hh[P].astype(np.float32)
    comb = bih_p + bhh_p
    bias4 = np.zeros((4, PB), np.float32)
    for j in range(NG):
        g = j * GFD
        bias4[j, 0:2 * HC] = comb[g:g + 2 * HC]                  # r|z combined
        bias4[j, 2 * HC:3 * HC] = bhh_p[g + 2 * HC:g + 3 * HC]   # hn bias
        bias4[j, 3 * HC:4 * HC] = bih_p[g + 2 * HC:g + 3 * HC]   # xn bias
    ind4 = np.zeros((4, 128), np.float32)
    for j in range(NG):
        ind4[j, 32 * j:32 * (j + 1)] = 1.0
    ident = np.eye(128, dtype=np.float32)
    bp = np.zeros((128, PB + 128), np.float32)
    bp[0:4, 0:PB] = bias4
    bp[0:4, PB:PB + 128] = ind4
    wpack = np.concatenate(
        [wih[0:128], wih[128:256]]
        + [whh[128 * c:128 * (c + 1)] for c in range(4)]
        + [ident, bp], axis=1)
    return {"wpack": np.ascontiguousarray(wpack)}


def host_blob(x, wpack, core):
    xt = host_prepare_x(x, core)                      # [256, T*BS]
    return np.ascontiguousarray(
        np.concatenate([xt[0:128], xt[128:256], wpack], axis=1))


def host_prepare_x(x, core):
    """Per-core transposed x: [256, T*BS], col = t*BS + b."""
    xs = x[core * BS:(core + 1) * BS]                # [BS, T, X]
    t = xs.shape[1]
    return np.ascontiguousarray(
        xs.transpose(2, 1, 0).reshape(X, t * BS)).astype(np.float32)


def host_post(out_core):
    """[112, 128] packed h' -> [BS, H]."""
    h = np.zeros((BS, H), np.float32)
    for j in range(NG):
        h[:, j * HC:(j + 1) * HC] = out_core[32 * j:32 * j + BS, :]
    return h


def build_kernel(T=T_FULL, CH=CH):
    """Per-core GRU program.

    Packed natural layout: batch rows at partitions 32j+b (h-chunk j,
    b<16); rows 32j+16..32j+32 are computed junk. One 2KB PSUM bank per
    step holds [r|z|hn|xn] preactivations: an M=128 K=4 indicator-matrix
    bias matmul (start=True) clears the bank and seeds biases for every
    partition, then x-side and recurrent matmuls accumulate on top
    (4 tile_position column groups in parallel, W_hh rhs N=384 per
    group). The 8-op elementwise tail spans all 128 partitions; h' is
    PE-transposed against an identity so its columns become next step's
    stationary lhsT chunks.
    """
    assert T % CH == 0 and CH % 2 == 0
    nc = bass.Bass("TRN2")

    WCOLS = 6 * 3 * H + 128 + PB + 128
    blob = nc.dram_tensor("blob", [128, 2 * T * BS + WCOLS], F32,
                          kind="ExternalInput")
    hout = nc.dram_tensor("hout", [112, HC], F32, kind="ExternalOutput")

    sig = mybir.ActivationFunctionType.Sigmoid
    tanh = mybir.ActivationFunctionType.Tanh

    with TileContext(nc) as tc:
        with (
            tc.tile_pool(name="consts", bufs=1) as cpool,
            tc.tile_pool(name="state", bufs=1) as spool,
            tc.tile_pool(name="xc", bufs=2) as xpool,
            tc.tile_pool(name="work", bufs=2) as wpool,
            tc.tile_pool(name="psumG", bufs=2, space="PSUM") as pgpool,
            tc.tile_pool(name="psumT", bufs=2, space="PSUM") as ptpool,
        ):
            # ---- one blob DMA: x + all weights (minimize DMA sems: the
            # For_i back-edge drain has a hard cap on sync waits) ----
            bl_sb = cpool.tile([128, 2 * T * BS + WCOLS], F32, tag="blob")
            nc.sync.dma_start(out=bl_sb[:], in_=blob[:, :])
            xbig = bl_sb[:, 0:2 * T * BS].rearrange("p (a w) -> p a w", a=2)
            wp_sb = bl_sb[:, 2 * T * BS:]
            wih0 = wp_sb[:, 0:3 * H]
            wih1 = wp_sb[:, 3 * H:6 * H]
            whh_k = [wp_sb[:, (6 + 3 * c) * H:(9 + 3 * c) * H] for c in range(4)]
            id_sb = wp_sb[:, 18 * H:18 * H + 128]
            b4_sb = wp_sb[0:4, 18 * H + 128:18 * H + 128 + PB]
            i4_sb = wp_sb[0:4, 18 * H + 128 + PB:18 * H + 256 + PB]

            # ---- persistent state (parity-indexed) ----
            hprev = [spool.tile([128, HC], F32, tag=f"hprev{p}", name=f"hprev{p}")
                     for p in range(2)]
            hT_sb = [spool.tile([128, 128], F32, tag=f"hT{p}", name=f"hT{p}")
                     for p in range(2)]
            # t=0 reads parity 1 (h(-1) == 0)
            nc.vector.memset(hprev[1][:], 0.0)
            nc.vector.memset(hT_sb[1][:], 0.0)

            def step(s, xc0, xc1):
                """Emit one timestep. s = step index within chunk."""
                p = s % 2
                sl = bass.ts(s, BS)  # lhsT cols for this step in x chunk
                pG = pgpool.tile([128, PB], F32, tag="pG")

                # --- bias start matmul: clears bank, writes all partitions ---
                nc.tensor.matmul(pG[:, :], i4_sb, b4_sb,
                                 start=True, stop=False, tile_position=(0, 0),
                                 skip_group_check=True)

                # --- input-side matmuls (prerun during prev tail) ---
                for j in range(NG):
                    o = slice(32 * j, 32 * j + BS)
                    g0 = j * GFD
                    nc.tensor.matmul(pG[o, 0:2 * HC], xc0[:, sl],
                                     wih0[:, g0:g0 + 2 * HC],
                                     start=False, stop=False,
                                     tile_position=(0, 32 * j),
                                     skip_group_check=True)
                    nc.tensor.matmul(pG[o, 0:2 * HC], xc1[:, sl],
                                     wih1[:, g0:g0 + 2 * HC],
                                     start=False, stop=False,
                                     tile_position=(0, 32 * j),
                                     skip_group_check=True)
                    nc.tensor.matmul(pG[o, 3 * HC:PB], xc0[:, sl],
                                     wih0[:, g0 + 2 * HC:g0 + GFD],
                                     start=False, stop=False,
                                     tile_position=(0, 32 * j),
                                     skip_group_check=True)
                    nc.tensor.matmul(pG[o, 3 * HC:PB], xc1[:, sl],
                                     wih1[:, g0 + 2 * HC:g0 + GFD],
                                     start=False, stop=False,
                                     tile_position=(0, 32 * j),
                                     skip_group_check=True)

                # --- transpose h(s-1) -> hT (PE), copy to SBUF ---
                pT = ptpool.tile([128, 128], F32, tag="pT")
                nc.tensor.transpose(pT[:, :], hprev[1 - p][:, :], id_sb)
                nc.scalar.copy(hT_sb[1 - p][:, :], pT[:, :])

                # --- recurrent matmuls: 4 k-chunk waves x 4 col groups ---
                for c in range(4):
                    for j in range(NG):
                        oo = slice(32 * j, 32 * (j + 1))
                        nc.tensor.matmul(
                            pG[oo, 0:GFD],
                            hT_sb[1 - p][:, 32 * c:32 * (c + 1)],
                            whh_k[c][:, j * GFD:(j + 1) * GFD],
                            start=False, stop=(c == 3 and j == NG - 1),
                            tile_position=(0, 32 * j),
                            skip_group_check=True)

                # --- elementwise tail ---
                rz = wpool.tile([128, 2 * HC], F32, tag="rz")
                m = wpool.tile([128, HC], F32, tag="m")
                a = wpool.tile([128, HC], F32, tag="a")
                n_t = wpool.tile([128, HC], F32, tag="n")
                w_t = wpool.tile([128, HC], F32, tag="w")
                q = wpool.tile([128, HC], F32, tag="q")
                wn = wpool.tile([128, HC], F32, tag="wn")

                nc.scalar.activation(rz[:], pG[:, 0:2 * HC], sig)
                nc.vector.tensor_tensor(m[:], rz[:, 0:HC], pG[:, 2 * HC:3 * HC],
                                        mybir.AluOpType.mult)
                nc.vector.tensor_tensor(a[:], m[:], pG[:, 3 * HC:PB],
                                        mybir.AluOpType.add)
                nc.scalar.activation(n_t[:], a[:], tanh)
                nc.scalar.activation(w_t[:], pG[:, HC:2 * HC], sig, scale=-1.0)
                nc.vector.tensor_tensor(q[:], rz[:, HC:2 * HC], hprev[1 - p][:, :],
                                        mybir.AluOpType.mult)
                nc.vector.tensor_tensor(wn[:], w_t[:], n_t[:],
                                        mybir.AluOpType.mult)
                nc.vector.tensor_tensor(hprev[p][:, :], wn[:], q[:],
                                        mybir.AluOpType.add)

            if T == CH:
                for s in range(CH):
                    step(s, xbig[:, 0, 0:CH * BS], xbig[:, 1, 0:CH * BS])
            else:
                with tc.For_i(0, T * BS, CH * BS,
                              hint_engines=tuple(mybir.ALL_ENGINES)) as iv:
                    # chunk copy resolves the dynamic offset (ldweights
                    # cannot take register offsets)
                    xc = xpool.tile([128, 2, CH * BS], F32, tag="xc")
                    nc.vector.tensor_copy(
                        xc[:, :, :], xbig[:, :, bass.ds(iv, CH * BS)])
                    for s in range(CH):
                        step(s, xc[:, 0, :], xc[:, 1, :])

            # final h lives in hprev[(T-1) % 2]
            nc.sync.dma_start(out=hout[:, :], in_=hprev[(T - 1) % 2][0:112, :])

    _cap_sync_waits(nc)
    return nc


def _cap_sync_waits(nc, cap=1):
    """Walrus codegen rejects >cap sync waits on one instruction.

    The TPB CTRL instruction structs have exactly ONE events slot, so a
    Drain/NoOp can encode at most one sync wait. The offenders are the
    For_i back-edge drain and loop-exit barrier NoOps, whose waits are
    redundant here: the loop body issues no DMAs, so every tile
    semaphore is incremented by an in-order engine queue, and the
    butterfly barrier that follows already guarantees all engines
    (hence all increments) are done. Keep the most load-bearing sem
    (DVE produces h'), drop the rest.
    """
    pref = {"DVE": 0, "Activation": 1, "PE": 2, "SP": 3}

    def rank(w):
        return pref.get(w.ant_name.split("_")[0], 9)

    for bb in nc.m.functions[0].blocks:
        for inst in bb.instructions:
            si = getattr(inst, "sync_info", None)
            if si and si.on_wait and len(si.on_wait) > cap:
                si.on_wait = sorted(si.on_wait, key=rank)[:cap]


_NC_CACHE = {}


def run(x, W_ih, W_hh, b_ih, b_hh, trace=False):
    from concourse.bass_utils import run_bass_kernel_spmd

    x = np.asarray(x, dtype=np.float32)
    W_ih = np.asarray(W_ih, dtype=np.float32)
    W_hh = np.asarray(W_hh, dtype=np.float32)
    b_ih = np.asarray(b_ih, dtype=np.float32)
    b_hh = np.asarray(b_hh, dtype=np.float32)

    key = (x.shape[1],)
    if key not in _NC_CACHE:
        _NC_CACHE[key] = build_kernel(T=x.shape[1])
    nc = _NC_CACHE[key]

    wts = host_prepare_weights(W_ih, W_hh, b_ih, b_hh)
    in_maps = [{"blob": host_blob(x, wts["wpack"], c)} for c in range(NCORES)]
    res = run_bass_kernel_spmd(nc, in_maps, list(range(NCORES)), trace=trace)
    h = np.zeros((B, H), np.float32)
    for c in range(NCORES):
        h[c * BS:(c + 1) * BS] = host_post(np.asarray(res.results[c]["hout"]))
    return h, res


def kernel(x, W_ih, W_hh, b_ih, b_hh):
    h, _ = run(x, W_ih, W_hh, b_ih, b_hh)
    return h
